# revision 36
# baseline (speedup 1.0000x reference)
"""Trainium2 Bass kernel for nn_Decoder_75892072120909 (sparse-attention decoder).

Self-contained: takes FULL inputs (as produced by the problem's setup_inputs),
runs an 8-core SPMD Bass kernel, returns the FULL output [2, 1920, 1024].

Sharding: 2 batches x 4 cores; each core owns 4 frame blocks (the last core of
a batch owns frames [11,12,13,14]; frame 11 is taken from the previous core so
every core runs the identical SPMD program). Each core also replicates the
tiny "non-frame" token trajectory (delim + dynamics tokens, 12 per block = 180
per batch) whose attention is the identity (those tokens attend only to
themselves), so no cross-core communication is needed.

On-core layout and schedule:
- Activations are feature-on-partition (hT [512, Ntok]); the residual stream
  stays fp32 on-chip, carried as 16*h (h0, wo and wproj are host-scaled by 16
  and the logit weights by 1/16) so the fp8 down-projection needs no extra
  rescale; LN is scale-invariant so nothing else changes.
- Matmul operands are bf16 (1 cycle/row at any tile size, half the weight
  DMA); the MLP down-projection runs fp8e4 DoubleRow (2 K-chunks per matmul):
  gelu writes fp8 uc-pairs that are exactly the DoubleRow moving operand.
  Weights are double-buffered so layer l+1's DMA overlaps layer l's compute.
- Attention output is accumulated token-major ([q, d+1] with an appended
  ones-column), so the softmax normalizer is a per-partition scalar: a [128,4]
  DVE reciprocal + per-partition tensor_scalar, then a PE transpose back to
  feature-major. The head loop is software-pipelined: scores(h) | o(h-2) |
  transposes(h-3), so the PE never sits on the exp/normalize dependencies.
- LayerNorm statistics are ones-vector matmuls (f32r); each LN's stat matmuls
  are fused into the tail of the previous phase (LN2 into the wo passes, the
  next layer's LN1 into the MLP passes) with the Ln/Exp+broadcast deferred so
  the ACT table set never thrashes (one switch to gelu and back per layer).
- V-chunk computation and the non-frame v transposes ride inside the QKV
  passes; non-frame tokens never issue queries; layer 7 skips all non-frame
  wo/LN2/MLP work and the logits ride layer 7's MLP passes.
- PSUM: psA(4 banks, rotating) + psM g0..g3; the QKV/wo evacuations are
  spread across all 8 banks so bank recycling never gates the PE.
"""

import sys
import numpy as np

for _p in ("/opt/trn_rl_repo", "/root/.axon_site/_ro/trn_rl_repo"):
    if _p not in sys.path:
        sys.path.append(_p)

import concourse.bass as bass
import concourse.tile as tile
from concourse import mybir
from concourse.bass_utils import run_bass_kernel_spmd
from concourse.masks import make_identity

# ---------------- problem constants (hardcoded) ----------------
F = 128           # frame tokens per block
T = 10            # dynamics tokens per block
BLK = F + T + 2   # 140
N = 15            # frame blocks
B = 2
W = 512
L = 8
H = 8
DH = 64
S = N * BLK       # 2100
NNF = N * (T + 2)  # 180 non-frame tokens per batch
NQ = 4 * F        # 512 frame-token queries per core
NTOK = NNF + NQ   # 692 tokens per core
EPS = 1e-5
NEG = -1e30
SCALE = 1.0 / np.sqrt(DH)
CORE_FRAMES = [[0, 1, 2, 3], [4, 5, 6, 7], [8, 9, 10, 11], [11, 12, 13, 14]]
F32 = mybir.dt.float32
F32R = mybir.dt.float32r
BF16 = mybir.dt.bfloat16
FP8 = mybir.dt.float8e4
AF = mybir.ActivationFunctionType
OP = mybir.AluOpType
# attention-aligned token chunks over the 692 on-core tokens
TCH = [(0, 128), (128, 180), (180, 308), (308, 436), (436, 564), (564, 692)]
TP = ((0, 346), (346, NTOK))  # moving-operand token passes


def build_program():
    nc = bass.Bass("TRN2", target_bir_lowering=False, debug=False, num_devices=8)

    h0 = nc.dram_tensor("h0", [W, NTOK], F32, kind="ExternalInput").ap()
    wqkvT_d = nc.dram_tensor("wqkvT", [L, W, 3 * W], BF16, kind="ExternalInput").ap()
    woT_d = nc.dram_tensor("woT", [L, W, W], BF16, kind="ExternalInput").ap()
    wfcT_d = nc.dram_tensor("wfcT", [L, W, 4 * W], BF16, kind="ExternalInput").ap()
    wprojT_d = nc.dram_tensor("wprojT", [L, 4 * W, W], FP8, kind="ExternalInput").ap()
    predT_d = nc.dram_tensor("predT", [W, 1024], F32, kind="ExternalInput").ap()
    mask_d = nc.dram_tensor("nfmask", [NNF, 512], F32, kind="ExternalInput").ap()
    cvec_d = nc.dram_tensor("cvec", [128, 12], F32, kind="ExternalInput").ap()
    crow_d = nc.dram_tensor("crow", [1, 128], F32, kind="ExternalInput").ap()
    out_d = nc.dram_tensor("logits", [NQ, 1024], F32, kind="ExternalOutput").ap()

    with tile.TileContext(nc) as tc:
        _build(tc, h0, wqkvT_d, woT_d, wfcT_d, wprojT_d, predT_d, mask_d,
               cvec_d, crow_d, out_d)

    from waitsplit_embedded import split_excess_waits
    split_excess_waits(nc)
    return nc


def _build(tc, h0, wqkvT_d, woT_d, wfcT_d, wprojT_d, predT_d, mask_d,
           cvec_d, crow_d, out_d):
    nc = tc.nc
    from contextlib import ExitStack
    ctx = ExitStack()

    def pool(name, bufs, **kw):
        return ctx.enter_context(tc.tile_pool(name=name, bufs=bufs, **kw))

    state = pool("state", 1)
    apool = pool("apool", 1)
    qkp = pool("qkp", 1)
    vp = pool("vp", 1)
    attp = pool("attp", 1)
    ep = pool("ep", 6)
    eop = pool("eop", 4)
    onp = pool("onp", 3)     # normalized per-head token-major attention out
    rcp = pool("rcp", 3)     # per-query softmax reciprocal columns
    up = pool("up", 3)
    sq = pool("sq", 4)
    mpool = pool("mpool", 1)
    sp = pool("sp", 1)
    wq_p = pool("wq", 2)
    wo_p = pool("wo", 2)
    wfc_p = pool("wfc", 2)
    wpr_p = pool("wpr", 2)
    prp = pool("prp", 1)
    cst = pool("cst", 1)
    lout = pool("lout", 2)

    # PSUM: 8 banks total. psA(mm)x4 + psM tags g0..g3 x1 = 8. g0/g1 double
    # as the LN broadcast banks and g2/g3 as the per-head token-major
    # attention output accumulators (phases don't overlap with the MLP
    # accumulation).
    psA = pool("psA", 4, space="PSUM")
    psM = pool("psM", 1, space="PSUM")

    # ---- constants ----
    ident = cst.tile([128, 128], BF16, name="ident")
    make_identity(nc, ident)
    ones_inv = cst.tile([128, 1], F32R, name="ones_inv")   # value 1/512
    nc.sync.dma_start(out=ones_inv, in_=cvec_d[:, 0:1].bitcast(F32R))
    ones_row = cst.tile([1, 128], F32R, name="ones_row")   # value 1.0
    nc.sync.dma_start(out=ones_row, in_=crow_d.bitcast(F32R))
    cvec_t = cst.tile([128, 12], F32, name="cvec_t")
    nc.sync.dma_start(out=cvec_t, in_=cvec_d)
    maskb_t = cst.tile([128, 2, 512], F32, name="maskb_t")
    nc.sync.dma_start(out=maskb_t[0:128, 0, :], in_=mask_d[0:128, :])
    nc.sync.dma_start(out=maskb_t[0:52, 1, :], in_=mask_d[128:180, :])

    # ---- persistent activations ----
    hT = state.tile([128, 4, NTOK], F32R, name="hT")
    h0r = h0.rearrange("(c p) t -> p c t", p=128).bitcast(F32R)
    nc.sync.dma_start(out=hT[:, :, 0:346], in_=h0r[:, :, 0:346])
    nc.sync.dma_start(out=hT[:, :, 346:NTOK], in_=h0r[:, :, 346:NTOK])
    qkT = qkp.tile([128, 8, NTOK], BF16, name="qkT")
    v_aug = vp.tile([128, 6, H, DH + 1], BF16, name="v_aug")
    for ci in range(6):
        nc.gpsimd.memset(v_aug[:, ci, :, DH:DH + 1], 1.0)
    attnT = attp.tile([128, 4, NTOK], BF16, name="attnT")

    # prefetch the (fp32) logit weights once; used only after the layer loop
    pred_t = prp.tile([128, 4, 1024], F32R, name="pred_t")

    aT = None  # set by layer-0 prologue below / end of each layer's mlp

    def emit_logits(ks):
        for k in ks:
            for nb in range(2):
                ps = psA.tile([128, 512], F32, tag="mm", name=f"lg{k}_{nb}")
                for c in range(4):
                    nc.tensor.matmul(ps, hT[:, c, NNF + 128 * k:NNF + 128 * k + 128],
                                     pred_t[:, c, 512 * nb:512 * nb + 512],
                                     start=(c == 0), stop=(c == 3))
                lo = lout.tile([128, 512], F32, tag="lo", name=f"lo{k}_{nb}")
                nc.vector.tensor_copy(out=lo, in_=ps)
                nc.sync.dma_start(
                    out=out_d[128 * k:128 * k + 128, 512 * nb:512 * nb + 512],
                    in_=lo)

    def bcast(row_ap, n_part, n_free, tag):
        """[1, n] f32r row -> [n_part, n_free] PSUM tile via K=1 matmul."""
        pb = psM.tile([128, 512], F32, tag=tag, name=f"pb_{tag}_{nc.next_id()}")
        nc.tensor.matmul(pb[0:n_part, 0:n_free], ones_row[:, 0:n_part], row_ap,
                         start=True, stop=True)
        return pb

    def ln_stats_mm(t0, t1):
        """Stats matmuls + DVE chain up to var. No ACT (table-set safe)."""
        n = t1 - t0
        mu_ps = psA.tile([1, 512], F32, tag="mm", name=f"mu{tc.nc.next_id()}")
        ms_ps = psA.tile([1, 512], F32, tag="mm", name=f"ms{tc.nc.next_id()}")
        for c in range(4):
            nc.tensor.matmul(mu_ps[:, 0:n], ones_inv, hT[:, c, t0:t1],
                             start=(c == 0), stop=(c == 3))
        for c in range(4):
            hsq = sq.tile([128, 346], F32R, tag="hsq", name=f"hsq{tc.nc.next_id()}")
            nc.vector.tensor_mul(hsq[:, 0:n], hT[:, c, t0:t1], hT[:, c, t0:t1])
            nc.tensor.matmul(ms_ps[:, 0:n], ones_inv, hsq[:, 0:n],
                             start=(c == 0), stop=(c == 3))
        mu = sp.tile([1, 346], F32R, tag=f"mu{t0}", name=f"muv{tc.nc.next_id()}")
        nc.vector.tensor_copy(out=mu[:, 0:n], in_=mu_ps[:, 0:n])
        musq = sp.tile([1, 346], F32, tag="musq", name=f"mq{tc.nc.next_id()}")
        nc.vector.tensor_mul(musq[:, 0:n], mu[:, 0:n], mu[:, 0:n])
        var = sp.tile([1, 346], F32, tag=f"var{t0}", name=f"var{tc.nc.next_id()}")
        nc.vector.tensor_tensor(out=var[:, 0:n], in0=ms_ps[:, 0:n],
                                in1=musq[:, 0:n], op=OP.subtract)
        return mu, var

    def ln_stats_act(t0, t1, mu, var, tag_mu, tag_rs):
        """rstd = exp(-0.5*ln(var+eps)) on the ln/exp table set + bcasts."""
        n = t1 - t0
        lnv = sp.tile([1, 346], F32, tag=f"lnv{t0}", name=f"lnv{tc.nc.next_id()}")
        nc.scalar.activation(lnv[:, 0:n], var[:, 0:n], AF.Ln,
                             bias=cvec_t[0:1, 2:3])
        rstd = sp.tile([1, 346], F32R, tag=f"rs{t0}", name=f"rsd{tc.nc.next_id()}")
        nc.scalar.activation(rstd[:, 0:n], lnv[:, 0:n], AF.Exp, scale=-0.5)
        mu_b = bcast(mu[:, 0:n], 128, n, tag_mu)
        rs_b = bcast(rstd[:, 0:n], 128, n, tag_rs)
        return mu_b, rs_b

    def ln_fin(dst, t0, t1, mu_b, rs_b, via_tmp=False):
        n = t1 - t0
        for c in range(4):
            if via_tmp:
                tmp = sq.tile([128, 346], BF16, tag="lnf",
                              name=f"lnf{tc.nc.next_id()}")
                nc.vector.tensor_tensor(out=tmp[:, 0:n], in0=hT[:, c, t0:t1],
                                        in1=mu_b[:, 0:n], op=OP.subtract)
                nc.vector.tensor_mul(dst[:, c, t0:t1], tmp[:, 0:n],
                                     rs_b[:, 0:n])
            else:
                nc.vector.tensor_tensor(out=dst[:, c, t0:t1],
                                        in0=hT[:, c, t0:t1],
                                        in1=mu_b[:, 0:n], op=OP.subtract)
                nc.vector.tensor_mul(dst[:, c, t0:t1], dst[:, c, t0:t1],
                                     rs_b[:, 0:n])

    def ln_all(dst, via_tmp=False):
        """dst <- LN(hT): both passes' stats first, then finalizes."""
        sm = [ln_stats_mm(t0, t1) for (t0, t1) in TP]
        for i, (t0, t1) in enumerate(TP):
            mu_b, rs_b = ln_stats_act(t0, t1, *sm[i], f"g{2 * i}", f"g{2 * i + 1}")
            ln_fin(dst, t0, t1, mu_b, rs_b, via_tmp=via_tmp)

    aT = apool.tile([128, 4, NTOK], BF16, tag="a", name="aT0")
    ln_all(aT)

    for l in range(L):
        wq_t = wq_p.tile([128, 4, 3 * W], BF16, tag="wq", name=f"wq{l}")
        nc.sync.dma_start(out=wq_t,
                          in_=wqkvT_d[l].rearrange("(c p) f -> p c f", p=128))
        wo_t = wo_p.tile([128, 4, W], BF16, tag="wo", name=f"wo{l}")
        nc.sync.dma_start(out=wo_t,
                          in_=woT_d[l].rearrange("(c p) f -> p c f", p=128))
        wfc_t = wfc_p.tile([128, 4, 4 * W], BF16, tag="wfc", name=f"wfc{l}")
        nc.sync.dma_start(out=wfc_t,
                          in_=wfcT_d[l].rearrange("(c p) f -> p c f", p=128))
        wpr_t = wpr_p.tile([128, 16, W], FP8, tag="wpr", name=f"wpr{l}")
        nc.sync.dma_start(out=wpr_t,
                          in_=wprojT_d[l].rearrange("(c p) f -> p c f", p=128))
        if l == 1:
            nc.sync.dma_start(out=pred_t,
                              in_=predT_d.rearrange("(c p) f -> p c f",
                                                    p=128).bitcast(F32R))

        def emit_v(cis):
            for ci in cis:
                t0, t1 = TCH[ci]
                rows = t1 - t0
                ps = psA.tile([128, 512], F32, tag="mm", name=f"v{l}_{ci}")
                for c in range(4):
                    nc.tensor.matmul(ps[0:rows, :], aT[:, c, t0:t1],
                                     wq_t[:, c, 1024:1536], start=(c == 0), stop=(c == 3))
                nc.scalar.activation(v_aug[0:rows, ci, :, 0:DH],
                                     ps[0:rows, :].rearrange("p (hh d) -> p hh d", hh=8),
                                     AF.Copy)
                if ci < 2:
                    tp_ps = psA.tile([128, 4, 128], BF16, tag="mm",
                                     name=f"tp{l}_{ci}")
                    for cc in range(4):
                        for hh in range(2):
                            nc.tensor.transpose(
                                tp_ps[64 * hh:64 * hh + 64, cc, 0:rows],
                                v_aug[0:rows, ci, 2 * cc + hh, 0:DH],
                                ident[0:rows, 0:rows])
                    nc.vector.tensor_copy(
                        out=attnT[:, :, 128 * ci:128 * ci + rows],
                        in_=tp_ps[:, :, 0:rows])

        # ---- QKV (aT was produced at the end of the previous layer) ----
        for t0, t1 in TP:
            for fc in range(8):
                # nf tokens never issue queries: q chunks (fc<4) skip them
                s0 = NNF if (fc < 4 and t0 == 0) else t0
                n = t1 - s0
                # spread across all 8 PSUM banks so evacuation never gates
                if fc < 4:
                    ps = psA.tile([128, 512], F32, tag="mm", name=f"qk{l}_{fc}_{t0}")
                else:
                    ps = psM.tile([128, 512], F32, tag=f"g{fc - 4}",
                                  name=f"qk{l}_{fc}_{t0}")
                for c in range(4):
                    nc.tensor.matmul(ps[:, 0:n], wq_t[:, c, 128 * fc:128 * fc + 128],
                                     aT[:, c, s0:t1], start=(c == 0), stop=(c == 3))
                # alternate evacuation engine so PSUM banks recycle faster
                if fc % 2 == 0:
                    nc.scalar.activation(qkT[:, fc, s0:t1], ps[:, 0:n], AF.Copy)
                else:
                    nc.vector.tensor_copy(out=qkT[:, fc, s0:t1], in_=ps[:, 0:n])
            emit_v((0, 1, 2) if t0 == 0 else (3, 4, 5))

        # ---- attention (per head, output accumulated token-major) ----
        tp_prev = [None]
        ESTASH = [None] * H
        pend_o = []
        pend_tp = []

        def emit_scores(h):
            r0 = 64 * (h % 2)
            qc, kc = h // 2, 4 + h // 2
            es = []
            for ci, rows in ((0, 128), (1, 52)):
                if ci == 0:
                    sc = psA.tile([128, 512], F32, tag="mm",
                                  name=f"snf{l}_{h}_{ci}")
                else:
                    sc = psM.tile([128, 512], F32,
                                  tag=("g0" if h % 2 == 0 else "g1"),
                                  name=f"snf{l}_{h}_{ci}")
                nc.tensor.matmul(sc[0:rows, :],
                                 qkT[r0:r0 + 64, kc, 128 * ci:128 * ci + rows],
                                 qkT[r0:r0 + 64, qc, NNF:NTOK],
                                 start=True, stop=True)
                nc.vector.tensor_add(out=sc[0:rows, :], in0=sc[0:rows, :],
                                     in1=maskb_t[0:rows, ci, :])
                e = ep.tile([128, 512], BF16, tag="e", name=f"e{l}_{h}_{ci}")
                nc.scalar.activation(e[0:rows, :], sc[0:rows, :], AF.Exp,
                                     scale=SCALE)
                es.append((e, rows))
            sc = psA.tile([128, 512], F32, tag="mm", name=f"sow{l}_{h}")
            for a in range(4):
                t0 = NNF + 128 * a
                nc.tensor.matmul(sc[:, 128 * a:128 * a + 128],
                                 qkT[r0:r0 + 64, kc, t0:t0 + 128],
                                 qkT[r0:r0 + 64, qc, t0:t0 + 128],
                                 start=(a == 0), stop=(a == 3))
            e2 = eop.tile([128, 512], BF16, tag="e2", name=f"e2{l}_{h}")
            nc.scalar.activation(e2, sc, AF.Exp, scale=SCALE)
            ESTASH[h] = (es, e2)

        def emit_o(h):
            es, e2 = ESTASH[h]
            o_tok = psM.tile([128, 4, DH + 1], F32,
                             tag=("g2" if h % 2 == 0 else "g3"),
                             name=f"ot{l}_{h}")
            for a in range(4):
                for ei, (e, rows) in enumerate(es):
                    nc.tensor.matmul(o_tok[:, a, :],
                                     e[0:rows, 128 * a:128 * a + 128],
                                     v_aug[0:rows, ei, h, :],
                                     start=(ei == 0), stop=False)
                nc.tensor.matmul(o_tok[:, a, :], e2[:, 128 * a:128 * a + 128],
                                 v_aug[:, 2 + a, h, :], start=False, stop=True)
            rc = rcp.tile([128, 4], F32, tag="rc", name=f"rc{l}_{h}")
            with nc.allow_low_precision(reason="softmax normalizer"):
                nc.vector.reciprocal(out=rc, in_=o_tok[:, :, DH])
            on = onp.tile([128, 4, DH], BF16, tag="on", name=f"on{l}_{h}")
            for a in range(4):
                nc.vector.tensor_scalar_mul(on[:, a, :], o_tok[:, a, 0:DH],
                                            rc[:, a:a + 1])
            pend_tp.append((h, on))

        def emit_tp(h, on):
            r0 = 64 * (h % 2)
            if h % 2 == 0:
                tp_pair = psA.tile([128, 4, 128], BF16, tag="mm",
                                   name=f"otp{l}_{h}")
                tp_prev[0] = tp_pair
            else:
                tp_pair = tp_prev[0]
            for a in range(4):
                nc.tensor.transpose(tp_pair[r0:r0 + 64, a, 0:128],
                                    on[:, a, :], ident)
            if h % 2 == 1:
                nc.vector.tensor_copy(out=attnT[:, h // 2, NNF:NTOK],
                                      in_=tp_pair.rearrange("p a q -> p (a q)"))

        # software pipeline: scores(h) | o(h-2) | transposes(h-3)
        for h in range(H):
            emit_scores(h)
            if h >= 2:
                emit_o(h - 2)
            if h >= 3:
                emit_tp(*pend_tp.pop(0))
        emit_o(H - 2)
        emit_tp(*pend_tp.pop(0))
        emit_o(H - 1)
        while pend_tp:
            emit_tp(*pend_tp.pop(0))

        # ---- O-proj + residual, interleaved with LN2 stats so the PE moves
        # straight from wo matmuls into stat matmuls while DVE finalizes ----

        def wo_pass(t0, t1):
            n = t1 - t0
            for fc in range(4):
                if fc < 2:
                    ps = psA.tile([128, 512], F32, tag="mm",
                                  name=f"op{l}_{fc}_{t0}")
                else:
                    ps = psM.tile([128, 512], F32, tag=f"g{fc}",
                                  name=f"op{l}_{fc}_{t0}")
                for c in range(4):
                    nc.tensor.matmul(ps[:, 0:n], wo_t[:, c, 128 * fc:128 * fc + 128],
                                     attnT[:, c, t0:t1], start=(c == 0), stop=(c == 3))
                nc.vector.tensor_add(out=hT[:, fc, t0:t1], in0=hT[:, fc, t0:t1],
                                     in1=ps[:, 0:n])

        TPl = ((NNF, 346), TP[1]) if l == L - 1 else TP
        wo_pass(*TPl[0])
        wo_pass(*TPl[1])
        sm2 = [ln_stats_mm(*TPl[0]), ln_stats_mm(*TPl[1])]
        mT = mpool.tile([128, 4, NTOK], BF16, tag="m", name=f"mT{l}")
        for i, (t0, t1) in enumerate(TPl):
            mu_b, rs_b = ln_stats_act(t0, t1, *sm2[i], f"g{2 * i}", f"g{2 * i + 1}")
            ln_fin(mT, t0, t1, mu_b, rs_b)
        # preload the gelu table while LN2 finalize / MLP1 matmuls run
        dmy = sp.tile([1, 346], F32, tag="var0", name=f"dmy{l}")
        nc.scalar.activation(dmy[0:1, 0:1], mT[0:1, 0, 0:1], AF.Gelu)
        if l < L - 1:
            aT_next = apool.tile([128, 4, NTOK], BF16, tag="a", name=f"aT{l + 1}")
        sm1 = [None, None]
        MP = (((NNF, 436), (436, NTOK)) if l == L - 1
              else ((0, 256), (256, 512), (512, NTOK)))
        for pi, (t0, t1) in enumerate(MP):
            n = t1 - t0
            # two down-proj accumulators, 2 output chunks each (g0: fc0/1,
            # g1: fc2/3) -- frees g2/g3 for the ups rotation below
            acc = [psM.tile([128, 512], F32, tag=f"g{i}",
                            name=f"acc{l}_{t0}_{i}") for i in range(4)]
            ug2s = [None] * 8
            for pr in range(9):
                if pr < 8:
                    # ups for uc pair (2*pr, 2*pr+1) share one PSUM tile so
                    # one gelu call covers both
                    ups = psA.tile([128, 2, 256], F32, tag="mm",
                                   name=f"u{l}_{t0}_{pr}")
                    for ui in range(2):
                        uc = 2 * pr + ui
                        for c in range(4):
                            nc.tensor.matmul(ups[:, ui, 0:n],
                                             wfc_t[:, c, 128 * uc:128 * uc + 128],
                                             mT[:, c, t0:t1],
                                             start=(c == 0), stop=(c == 3))
                    ug2 = up.tile([128, 2, 256], FP8, tag="ug",
                                  name=f"ug{l}_{t0}_{pr}")
                    nc.scalar.activation(ug2[:, :, 0:n], ups[:, :, 0:n],
                                         AF.Gelu, scale=1.0 / 16.0)
                    ug2s[pr] = ug2
                if pr >= 1:
                    p = pr - 1
                    for fc in range(4):
                        nc.tensor.matmul(acc[fc][:, 0:n],
                                         wpr_t[:, 2 * p:2 * p + 2,
                                               128 * fc:128 * fc + 128],
                                         ug2s[p][:, :, 0:n],
                                         start=(p == 0), stop=(p == 7),
                                         perf_mode=mybir.MatmulPerfMode.DoubleRow)
            for fc in range(4):
                nc.vector.tensor_add(out=hT[:, fc, t0:t1], in0=hT[:, fc, t0:t1],
                                     in1=acc[fc][:, 0:n])
            if l < L - 1 and pi >= 1:
                # next layer's LN1 stat matmuls ride the tail of this pass --
                # TP[0] (tokens 0:346) is final once MP pass 1 (through 512)
                # has landed, TP[1] after the last pass. (No ACT here, so the
                # gelu table set is undisturbed.)
                sm1[pi - 1] = ln_stats_mm(*TP[pi - 1])
            if l == L - 1:
                # final-layer logits ride each pass (tokens are final here)
                emit_logits((0, 1) if pi == 0 else (2, 3))
        if l < L - 1:
            # Ln/Exp both live on the exp table set: this also switches the
            # table back for the next layer's attention exponentials
            for pi, (t0, t1) in enumerate(TP):
                mu_b, rs_b = ln_stats_act(t0, t1, *sm1[pi],
                                          f"g{2 * pi}", f"g{2 * pi + 1}")
                ln_fin(aT_next, t0, t1, mu_b, rs_b)
            aT = aT_next

    ctx.close()


# (waitsplit embedded so kernel.py is self-contained)
import types as _types
waitsplit_embedded = _types.ModuleType("waitsplit_embedded")


def _split_excess_waits(nc):
    n_split = 0
    for fn in nc.m.functions:
        for bb in fn.blocks:
            insts = list(bb.instructions)
            new_list = []
            changed = False
            for inst in insts:
                si = getattr(inst, "sync_info", None)
                waits = list(si.on_wait) if si is not None and si.on_wait else []
                cap = 2 if isinstance(inst, mybir.InstEventSemaphore) else 1
                if len(waits) > cap:
                    changed = True
                    keep = waits[-cap:]
                    for w in waits[:-cap]:
                        n_split += 1
                        nop = mybir.InstNoOp(
                            name=f"WSPLIT-{n_split}-{inst.name}",
                            engine=inst.engine,
                            ins=[], outs=[],
                            sync_info=mybir.SyncInfo(on_wait=[w], on_update=[]),
                        )
                        try:
                            nop.bass_nofuse = True
                        except Exception:
                            pass
                        new_list.append(nop)
                    inst.sync_info = mybir.SyncInfo(on_wait=keep,
                                                    on_update=list(si.on_update))
                new_list.append(inst)
            if changed:
                try:
                    bb.instructions = new_list
                except Exception:
                    bb.instructions.clear()
                    bb.instructions.extend(new_list)
    return n_split


waitsplit_embedded.split_excess_waits = _split_excess_waits
sys.modules["waitsplit_embedded"] = waitsplit_embedded


# ---------------- host side ----------------

def _sinusoidal_pos_emb(n_pos, d, n=10000.0):
    pos = np.arange(n_pos, dtype=np.float32)[:, None]
    den = np.power(n, 2.0 * np.arange(d // 2, dtype=np.float32) / d).astype(np.float32)
    emb = np.zeros((n_pos, d), dtype=np.float32)
    emb[:, 0::2] = np.sin(pos / den)
    emb[:, 1::2] = np.cos(pos / den)
    return emb


_PROG = None


def kernel(**inputs):
    global _PROG
    import ml_dtypes
    bf16 = ml_dtypes.bfloat16
    x = np.ascontiguousarray(np.asarray(inputs["x"], dtype=np.float32))
    f = np.ascontiguousarray(np.asarray(inputs["f"], dtype=np.float32))
    delim = np.asarray(inputs["frame_delim"], dtype=np.float32)
    wqkv = np.asarray(inputs["wqkv"], dtype=np.float32)
    wo = np.asarray(inputs["wo"], dtype=np.float32)
    wfc = np.asarray(inputs["wfc"], dtype=np.float32)
    wproj = np.asarray(inputs["wproj"], dtype=np.float32)
    pred_w = np.asarray(inputs["pred_w"], dtype=np.float32)

    # this kernel folds away the (identity) LN affine and (zero) biases;
    # verify that assumption against the actual inputs
    assert np.all(np.asarray(inputs["ln1_g"]) == 1), "nonconst ln1_g"
    assert np.all(np.asarray(inputs["ln2_g"]) == 1), "nonconst ln2_g"
    assert np.all(np.asarray(inputs["ln1_b"]) == 0), "nonzero ln1_b"
    assert np.all(np.asarray(inputs["ln2_b"]) == 0), "nonzero ln2_b"
    for bname in ("bqkv", "bo", "bfc", "bproj"):
        assert np.all(np.asarray(inputs[bname]) == 0), f"nonzero {bname}"

    d2 = np.broadcast_to(delim, (B, N, 1, W))
    fx = np.concatenate([x, d2, f, d2], axis=-2).reshape(B, S, W)
    fx = fx + _sinusoidal_pos_emb(S, W)[None]

    nf_idx = (np.arange(N)[:, None] * BLK + (F + np.arange(T + 2))[None, :]).reshape(-1)
    jj = np.arange(NNF) // (T + 2)
    rr = np.arange(NNF) % (T + 2)
    mask = np.full((NNF, N), NEG, np.float32)
    for i in range(N):
        allowed = ((rr <= T) & (jj <= i)) | ((rr == T + 1) & (jj == i - 1))
        mask[allowed, i] = 0.0

    fp8 = ml_dtypes.float8_e4m3
    # residual stream is carried as 16*h on-chip: h0 and the two residual-
    # producing projections (wo, wproj) are scaled by 16, and the logit
    # weights by 1/16. LN is scale-invariant, so everything else is unchanged.
    wqkvT = np.ascontiguousarray(wqkv.transpose(0, 2, 1).astype(bf16))
    woT = np.ascontiguousarray((wo * 16.0).transpose(0, 2, 1).astype(bf16))
    # wfc additionally x16 purely for fp8 range (folded back in gelu's scale)
    wfcT = np.ascontiguousarray((wfc * 16.0).transpose(0, 2, 1).astype(bf16))
    wprojT = np.ascontiguousarray((wproj * 16.0).transpose(0, 2, 1).astype(fp8))
    predT = np.ascontiguousarray(pred_w.T / 16.0)

    cvec = np.zeros((128, 12), np.float32)
    cvec[:, 0] = 1.0 / W
    cvec[:, 1] = 1.0
    cvec[:, 2] = EPS
    cvec[:, 4:12] = 1.0
    crow = np.ones((1, 128), np.float32)

    if _PROG is None:
        _PROG = build_program()
    nc = _PROG

    in_maps = []
    for c in range(8):
        b, slot = c // 4, c % 4
        cf = CORE_FRAMES[slot]
        fr_idx = np.concatenate([np.arange(i * BLK, i * BLK + F) for i in cf])
        tok = np.concatenate([nf_idx, fr_idx])
        h0T = np.ascontiguousarray(fx[b, tok, :].T * 16.0)
        in_maps.append({
            "h0": h0T,
            "wqkvT": wqkvT, "woT": woT, "wfcT": wfcT, "wprojT": wprojT,
            "predT": predT,
            "nfmask": np.ascontiguousarray(np.repeat(mask[:, cf], F, axis=1)),
            "cvec": cvec, "crow": crow,
        })

    res = run_bass_kernel_spmd(nc, in_maps, list(range(8)))

    out = np.zeros((B, N, F, 1024), np.float32)
    for c in range(8):
        b, slot = c // 4, c % 4
        lo = res.results[c]["logits"].reshape(4, F, 1024)
        for si, i in enumerate(CORE_FRAMES[slot]):
            if slot == 3 and si == 0:
                continue
            out[b, i] = lo[si]
    return out.reshape(B, N * F, 1024)


# revision 37
# speedup vs baseline: 1.0005x; 1.0005x over previous
"""Trainium2 Bass kernel for nn_Decoder_75892072120909 (sparse-attention decoder).

Self-contained: takes FULL inputs (as produced by the problem's setup_inputs),
runs an 8-core SPMD Bass kernel, returns the FULL output [2, 1920, 1024].

Sharding: 2 batches x 4 cores; each core owns 4 frame blocks (the last core of
a batch owns frames [11,12,13,14]; frame 11 is taken from the previous core so
every core runs the identical SPMD program). Each core also replicates the
tiny "non-frame" token trajectory (delim + dynamics tokens, 12 per block = 180
per batch) whose attention is the identity (those tokens attend only to
themselves), so no cross-core communication is needed.

On-core layout and schedule:
- Activations are feature-on-partition (hT [512, Ntok]); the residual stream
  stays fp32 on-chip, carried as 16*h (h0, wo and wproj are host-scaled by 16
  and the logit weights by 1/16) so the fp8 down-projection needs no extra
  rescale; LN is scale-invariant so nothing else changes.
- Matmul operands are bf16 (1 cycle/row at any tile size, half the weight
  DMA); the MLP down-projection runs fp8e4 DoubleRow (2 K-chunks per matmul):
  gelu writes fp8 uc-pairs that are exactly the DoubleRow moving operand.
  Weights are double-buffered so layer l+1's DMA overlaps layer l's compute.
- Attention output is accumulated token-major ([q, d+1] with an appended
  ones-column), so the softmax normalizer is a per-partition scalar: a [128,4]
  DVE reciprocal + per-partition tensor_scalar, then a PE transpose back to
  feature-major. The head loop is software-pipelined: scores(h) | o(h-2) |
  transposes(h-3), so the PE never sits on the exp/normalize dependencies.
- LayerNorm statistics are ones-vector matmuls (f32r); each LN's stat matmuls
  are fused into the tail of the previous phase (LN2 into the wo passes, the
  next layer's LN1 into the MLP passes) with the Ln/Exp+broadcast deferred so
  the ACT table set never thrashes (one switch to gelu and back per layer).
- V-chunk computation and the non-frame v transposes ride inside the QKV
  passes; non-frame tokens never issue queries; layer 7 skips all non-frame
  wo/LN2/MLP work and the logits ride layer 7's MLP passes.
- PSUM: psA(4 banks, rotating) + psM g0..g3; the QKV/wo evacuations are
  spread across all 8 banks so bank recycling never gates the PE.
"""

import sys
import numpy as np

for _p in ("/opt/trn_rl_repo", "/root/.axon_site/_ro/trn_rl_repo"):
    if _p not in sys.path:
        sys.path.append(_p)

import concourse.bass as bass
import concourse.tile as tile
from concourse import mybir
from concourse.bass_utils import run_bass_kernel_spmd
from concourse.masks import make_identity

# ---------------- problem constants (hardcoded) ----------------
F = 128           # frame tokens per block
T = 10            # dynamics tokens per block
BLK = F + T + 2   # 140
N = 15            # frame blocks
B = 2
W = 512
L = 8
H = 8
DH = 64
S = N * BLK       # 2100
NNF = N * (T + 2)  # 180 non-frame tokens per batch
NQ = 4 * F        # 512 frame-token queries per core
NTOK = NNF + NQ   # 692 tokens per core
EPS = 1e-5
NEG = -1e30
SCALE = 1.0 / np.sqrt(DH)
CORE_FRAMES = [[0, 1, 2, 3], [4, 5, 6, 7], [8, 9, 10, 11], [11, 12, 13, 14]]
F32 = mybir.dt.float32
F32R = mybir.dt.float32r
BF16 = mybir.dt.bfloat16
FP8 = mybir.dt.float8e4
AF = mybir.ActivationFunctionType
OP = mybir.AluOpType
# attention-aligned token chunks over the 692 on-core tokens
TCH = [(0, 128), (128, 180), (180, 308), (308, 436), (436, 564), (564, 692)]
TP = ((0, 346), (346, NTOK))  # moving-operand token passes


def build_program():
    nc = bass.Bass("TRN2", target_bir_lowering=False, debug=False, num_devices=8)

    h0 = nc.dram_tensor("h0", [W, NTOK], F32, kind="ExternalInput").ap()
    wqkvT_d = nc.dram_tensor("wqkvT", [L, W, 3 * W], BF16, kind="ExternalInput").ap()
    woT_d = nc.dram_tensor("woT", [L, W, W], BF16, kind="ExternalInput").ap()
    wfcT_d = nc.dram_tensor("wfcT", [L, W, 4 * W], BF16, kind="ExternalInput").ap()
    wprojT_d = nc.dram_tensor("wprojT", [L, 4 * W, W], FP8, kind="ExternalInput").ap()
    predT_d = nc.dram_tensor("predT", [W, 1024], F32, kind="ExternalInput").ap()
    mask_d = nc.dram_tensor("nfmask", [NNF, 512], F32, kind="ExternalInput").ap()
    cvec_d = nc.dram_tensor("cvec", [128, 12], F32, kind="ExternalInput").ap()
    crow_d = nc.dram_tensor("crow", [1, 128], F32, kind="ExternalInput").ap()
    out_d = nc.dram_tensor("logits", [NQ, 1024], F32, kind="ExternalOutput").ap()

    with tile.TileContext(nc) as tc:
        _build(tc, h0, wqkvT_d, woT_d, wfcT_d, wprojT_d, predT_d, mask_d,
               cvec_d, crow_d, out_d)

    from waitsplit_embedded import split_excess_waits
    split_excess_waits(nc)
    return nc


def _build(tc, h0, wqkvT_d, woT_d, wfcT_d, wprojT_d, predT_d, mask_d,
           cvec_d, crow_d, out_d):
    nc = tc.nc
    from contextlib import ExitStack
    ctx = ExitStack()

    def pool(name, bufs, **kw):
        return ctx.enter_context(tc.tile_pool(name=name, bufs=bufs, **kw))

    state = pool("state", 1)
    apool = pool("apool", 1)
    qkp = pool("qkp", 1)
    vp = pool("vp", 1)
    attp = pool("attp", 1)
    ep = pool("ep", 6)
    eop = pool("eop", 4)
    onp = pool("onp", 3)     # normalized per-head token-major attention out
    rcp = pool("rcp", 3)     # per-query softmax reciprocal columns
    up = pool("up", 3)
    sq = pool("sq", 4)
    mpool = pool("mpool", 1)
    sp = pool("sp", 1)
    wq_p = pool("wq", 2)
    wo_p = pool("wo", 2)
    wfc_p = pool("wfc", 2)
    wpr_p = pool("wpr", 2)
    prp = pool("prp", 1)
    cst = pool("cst", 1)
    lout = pool("lout", 2)

    # PSUM: 8 banks total. psA(mm)x4 + psM tags g0..g3 x1 = 8. g0/g1 double
    # as the LN broadcast banks and g2/g3 as the per-head token-major
    # attention output accumulators (phases don't overlap with the MLP
    # accumulation).
    psA = pool("psA", 4, space="PSUM")
    psM = pool("psM", 1, space="PSUM")

    # ---- constants ----
    ident = cst.tile([128, 128], BF16, name="ident")
    make_identity(nc, ident)
    ones_inv = cst.tile([128, 1], F32R, name="ones_inv")   # value 1/512
    nc.sync.dma_start(out=ones_inv, in_=cvec_d[:, 0:1].bitcast(F32R))
    ones_row = cst.tile([1, 128], F32R, name="ones_row")   # value 1.0
    nc.sync.dma_start(out=ones_row, in_=crow_d.bitcast(F32R))
    cvec_t = cst.tile([128, 12], F32, name="cvec_t")
    nc.sync.dma_start(out=cvec_t, in_=cvec_d)
    maskb_t = cst.tile([128, 2, 512], F32, name="maskb_t")
    nc.sync.dma_start(out=maskb_t[0:128, 0, :], in_=mask_d[0:128, :])
    nc.sync.dma_start(out=maskb_t[0:52, 1, :], in_=mask_d[128:180, :])

    # ---- persistent activations ----
    hT = state.tile([128, 4, NTOK], F32R, name="hT")
    h0r = h0.rearrange("(c p) t -> p c t", p=128).bitcast(F32R)
    nc.sync.dma_start(out=hT[:, :, 0:346], in_=h0r[:, :, 0:346])
    nc.sync.dma_start(out=hT[:, :, 346:NTOK], in_=h0r[:, :, 346:NTOK])
    qkT = qkp.tile([128, 8, NTOK], BF16, name="qkT")
    v_aug = vp.tile([128, 6, H, DH + 1], BF16, name="v_aug")
    for ci in range(6):
        nc.gpsimd.memset(v_aug[:, ci, :, DH:DH + 1], 1.0)
    attnT = attp.tile([128, 4, NTOK], BF16, name="attnT")

    # prefetch the (fp32) logit weights once; used only after the layer loop
    pred_t = prp.tile([128, 4, 1024], F32R, name="pred_t")

    aT = None  # set by layer-0 prologue below / end of each layer's mlp

    def emit_logits(ks):
        for k in ks:
            for nb in range(2):
                ps = psA.tile([128, 512], F32, tag="mm", name=f"lg{k}_{nb}")
                for c in range(4):
                    nc.tensor.matmul(ps, hT[:, c, NNF + 128 * k:NNF + 128 * k + 128],
                                     pred_t[:, c, 512 * nb:512 * nb + 512],
                                     start=(c == 0), stop=(c == 3))
                lo = lout.tile([128, 512], F32, tag="lo", name=f"lo{k}_{nb}")
                nc.vector.tensor_copy(out=lo, in_=ps)
                nc.sync.dma_start(
                    out=out_d[128 * k:128 * k + 128, 512 * nb:512 * nb + 512],
                    in_=lo)

    def bcast(row_ap, n_part, n_free, tag):
        """[1, n] f32r row -> [n_part, n_free] PSUM tile via K=1 matmul."""
        pb = psM.tile([128, 512], F32, tag=tag, name=f"pb_{tag}_{nc.next_id()}")
        nc.tensor.matmul(pb[0:n_part, 0:n_free], ones_row[:, 0:n_part], row_ap,
                         start=True, stop=True)
        return pb

    def ln_stats_mm(t0, t1):
        """Stats matmuls + DVE chain up to var. No ACT (table-set safe)."""
        n = t1 - t0
        mu_ps = psA.tile([1, 512], F32, tag="mm", name=f"mu{tc.nc.next_id()}")
        ms_ps = psA.tile([1, 512], F32, tag="mm", name=f"ms{tc.nc.next_id()}")
        for c in range(4):
            nc.tensor.matmul(mu_ps[:, 0:n], ones_inv, hT[:, c, t0:t1],
                             start=(c == 0), stop=(c == 3))
        for c in range(4):
            hsq = sq.tile([128, 346], F32R, tag="hsq", name=f"hsq{tc.nc.next_id()}")
            # split squares across DVE and the (otherwise idle) gpsimd so the
            # serial stats chain doesn't sit behind DVE's residual/finalize ops
            eng = nc.vector if c < 2 else nc.gpsimd
            eng.tensor_mul(hsq[:, 0:n], hT[:, c, t0:t1], hT[:, c, t0:t1])
            nc.tensor.matmul(ms_ps[:, 0:n], ones_inv, hsq[:, 0:n],
                             start=(c == 0), stop=(c == 3))
        mu = sp.tile([1, 346], F32R, tag=f"mu{t0}", name=f"muv{tc.nc.next_id()}")
        nc.vector.tensor_copy(out=mu[:, 0:n], in_=mu_ps[:, 0:n])
        musq = sp.tile([1, 346], F32, tag="musq", name=f"mq{tc.nc.next_id()}")
        nc.vector.tensor_mul(musq[:, 0:n], mu[:, 0:n], mu[:, 0:n])
        var = sp.tile([1, 346], F32, tag=f"var{t0}", name=f"var{tc.nc.next_id()}")
        nc.vector.tensor_tensor(out=var[:, 0:n], in0=ms_ps[:, 0:n],
                                in1=musq[:, 0:n], op=OP.subtract)
        return mu, var

    def ln_stats_act(t0, t1, mu, var, tag_mu, tag_rs):
        """rstd = exp(-0.5*ln(var+eps)) on the ln/exp table set + bcasts."""
        n = t1 - t0
        lnv = sp.tile([1, 346], F32, tag=f"lnv{t0}", name=f"lnv{tc.nc.next_id()}")
        nc.scalar.activation(lnv[:, 0:n], var[:, 0:n], AF.Ln,
                             bias=cvec_t[0:1, 2:3])
        rstd = sp.tile([1, 346], F32R, tag=f"rs{t0}", name=f"rsd{tc.nc.next_id()}")
        nc.scalar.activation(rstd[:, 0:n], lnv[:, 0:n], AF.Exp, scale=-0.5)
        mu_b = bcast(mu[:, 0:n], 128, n, tag_mu)
        rs_b = bcast(rstd[:, 0:n], 128, n, tag_rs)
        return mu_b, rs_b

    def ln_fin(dst, t0, t1, mu_b, rs_b, via_tmp=False):
        n = t1 - t0
        for c in range(4):
            if via_tmp:
                tmp = sq.tile([128, 346], BF16, tag="lnf",
                              name=f"lnf{tc.nc.next_id()}")
                nc.vector.tensor_tensor(out=tmp[:, 0:n], in0=hT[:, c, t0:t1],
                                        in1=mu_b[:, 0:n], op=OP.subtract)
                nc.vector.tensor_mul(dst[:, c, t0:t1], tmp[:, 0:n],
                                     rs_b[:, 0:n])
            else:
                nc.vector.tensor_tensor(out=dst[:, c, t0:t1],
                                        in0=hT[:, c, t0:t1],
                                        in1=mu_b[:, 0:n], op=OP.subtract)
                nc.vector.tensor_mul(dst[:, c, t0:t1], dst[:, c, t0:t1],
                                     rs_b[:, 0:n])

    def ln_all(dst, via_tmp=False):
        """dst <- LN(hT): both passes' stats first, then finalizes."""
        sm = [ln_stats_mm(t0, t1) for (t0, t1) in TP]
        for i, (t0, t1) in enumerate(TP):
            mu_b, rs_b = ln_stats_act(t0, t1, *sm[i], f"g{2 * i}", f"g{2 * i + 1}")
            ln_fin(dst, t0, t1, mu_b, rs_b, via_tmp=via_tmp)

    aT = apool.tile([128, 4, NTOK], BF16, tag="a", name="aT0")
    ln_all(aT)

    for l in range(L):
        wq_t = wq_p.tile([128, 4, 3 * W], BF16, tag="wq", name=f"wq{l}")
        nc.sync.dma_start(out=wq_t,
                          in_=wqkvT_d[l].rearrange("(c p) f -> p c f", p=128))
        wo_t = wo_p.tile([128, 4, W], BF16, tag="wo", name=f"wo{l}")
        nc.sync.dma_start(out=wo_t,
                          in_=woT_d[l].rearrange("(c p) f -> p c f", p=128))
        wfc_t = wfc_p.tile([128, 4, 4 * W], BF16, tag="wfc", name=f"wfc{l}")
        nc.sync.dma_start(out=wfc_t,
                          in_=wfcT_d[l].rearrange("(c p) f -> p c f", p=128))
        wpr_t = wpr_p.tile([128, 16, W], FP8, tag="wpr", name=f"wpr{l}")
        nc.sync.dma_start(out=wpr_t,
                          in_=wprojT_d[l].rearrange("(c p) f -> p c f", p=128))
        if l == 1:
            nc.sync.dma_start(out=pred_t,
                              in_=predT_d.rearrange("(c p) f -> p c f",
                                                    p=128).bitcast(F32R))

        def emit_v(cis):
            for ci in cis:
                t0, t1 = TCH[ci]
                rows = t1 - t0
                ps = psA.tile([128, 512], F32, tag="mm", name=f"v{l}_{ci}")
                for c in range(4):
                    nc.tensor.matmul(ps[0:rows, :], aT[:, c, t0:t1],
                                     wq_t[:, c, 1024:1536], start=(c == 0), stop=(c == 3))
                nc.scalar.activation(v_aug[0:rows, ci, :, 0:DH],
                                     ps[0:rows, :].rearrange("p (hh d) -> p hh d", hh=8),
                                     AF.Copy)
                if ci < 2:
                    tp_ps = psA.tile([128, 4, 128], BF16, tag="mm",
                                     name=f"tp{l}_{ci}")
                    for cc in range(4):
                        for hh in range(2):
                            nc.tensor.transpose(
                                tp_ps[64 * hh:64 * hh + 64, cc, 0:rows],
                                v_aug[0:rows, ci, 2 * cc + hh, 0:DH],
                                ident[0:rows, 0:rows])
                    nc.vector.tensor_copy(
                        out=attnT[:, :, 128 * ci:128 * ci + rows],
                        in_=tp_ps[:, :, 0:rows])

        # ---- QKV (aT was produced at the end of the previous layer) ----
        for t0, t1 in TP:
            for fc in range(8):
                # nf tokens never issue queries: q chunks (fc<4) skip them
                s0 = NNF if (fc < 4 and t0 == 0) else t0
                n = t1 - s0
                # spread across all 8 PSUM banks so evacuation never gates
                if fc < 4:
                    ps = psA.tile([128, 512], F32, tag="mm", name=f"qk{l}_{fc}_{t0}")
                else:
                    ps = psM.tile([128, 512], F32, tag=f"g{fc - 4}",
                                  name=f"qk{l}_{fc}_{t0}")
                for c in range(4):
                    nc.tensor.matmul(ps[:, 0:n], wq_t[:, c, 128 * fc:128 * fc + 128],
                                     aT[:, c, s0:t1], start=(c == 0), stop=(c == 3))
                # alternate evacuation engine so PSUM banks recycle faster
                if fc % 2 == 0:
                    nc.scalar.activation(qkT[:, fc, s0:t1], ps[:, 0:n], AF.Copy)
                else:
                    nc.vector.tensor_copy(out=qkT[:, fc, s0:t1], in_=ps[:, 0:n])
            emit_v((0, 1, 2) if t0 == 0 else (3, 4, 5))

        # ---- attention (per head, output accumulated token-major) ----
        tp_prev = [None]
        ESTASH = [None] * H
        pend_o = []
        pend_tp = []

        def emit_scores(h):
            r0 = 64 * (h % 2)
            qc, kc = h // 2, 4 + h // 2
            es = []
            for ci, rows in ((0, 128), (1, 52)):
                if ci == 0:
                    sc = psA.tile([128, 512], F32, tag="mm",
                                  name=f"snf{l}_{h}_{ci}")
                else:
                    sc = psM.tile([128, 512], F32,
                                  tag=("g0" if h % 2 == 0 else "g1"),
                                  name=f"snf{l}_{h}_{ci}")
                nc.tensor.matmul(sc[0:rows, :],
                                 qkT[r0:r0 + 64, kc, 128 * ci:128 * ci + rows],
                                 qkT[r0:r0 + 64, qc, NNF:NTOK],
                                 start=True, stop=True)
                nc.vector.tensor_add(out=sc[0:rows, :], in0=sc[0:rows, :],
                                     in1=maskb_t[0:rows, ci, :])
                e = ep.tile([128, 512], BF16, tag="e", name=f"e{l}_{h}_{ci}")
                nc.scalar.activation(e[0:rows, :], sc[0:rows, :], AF.Exp,
                                     scale=SCALE)
                es.append((e, rows))
            sc = psA.tile([128, 512], F32, tag="mm", name=f"sow{l}_{h}")
            for a in range(4):
                t0 = NNF + 128 * a
                nc.tensor.matmul(sc[:, 128 * a:128 * a + 128],
                                 qkT[r0:r0 + 64, kc, t0:t0 + 128],
                                 qkT[r0:r0 + 64, qc, t0:t0 + 128],
                                 start=(a == 0), stop=(a == 3))
            e2 = eop.tile([128, 512], BF16, tag="e2", name=f"e2{l}_{h}")
            nc.scalar.activation(e2, sc, AF.Exp, scale=SCALE)
            ESTASH[h] = (es, e2)

        def emit_o(h):
            es, e2 = ESTASH[h]
            o_tok = psM.tile([128, 4, DH + 1], F32,
                             tag=("g2" if h % 2 == 0 else "g3"),
                             name=f"ot{l}_{h}")
            for a in range(4):
                for ei, (e, rows) in enumerate(es):
                    nc.tensor.matmul(o_tok[:, a, :],
                                     e[0:rows, 128 * a:128 * a + 128],
                                     v_aug[0:rows, ei, h, :],
                                     start=(ei == 0), stop=False)
                nc.tensor.matmul(o_tok[:, a, :], e2[:, 128 * a:128 * a + 128],
                                 v_aug[:, 2 + a, h, :], start=False, stop=True)
            rc = rcp.tile([128, 4], F32, tag="rc", name=f"rc{l}_{h}")
            with nc.allow_low_precision(reason="softmax normalizer"):
                nc.vector.reciprocal(out=rc, in_=o_tok[:, :, DH])
            on = onp.tile([128, 4, DH], BF16, tag="on", name=f"on{l}_{h}")
            for a in range(4):
                nc.vector.tensor_scalar_mul(on[:, a, :], o_tok[:, a, 0:DH],
                                            rc[:, a:a + 1])
            pend_tp.append((h, on))

        def emit_tp(h, on):
            r0 = 64 * (h % 2)
            if h % 2 == 0:
                tp_pair = psA.tile([128, 4, 128], BF16, tag="mm",
                                   name=f"otp{l}_{h}")
                tp_prev[0] = tp_pair
            else:
                tp_pair = tp_prev[0]
            for a in range(4):
                nc.tensor.transpose(tp_pair[r0:r0 + 64, a, 0:128],
                                    on[:, a, :], ident)
            if h % 2 == 1:
                nc.vector.tensor_copy(out=attnT[:, h // 2, NNF:NTOK],
                                      in_=tp_pair.rearrange("p a q -> p (a q)"))

        # software pipeline: scores(h) | o(h-2) | transposes(h-3)
        for h in range(H):
            emit_scores(h)
            if h >= 2:
                emit_o(h - 2)
            if h >= 3:
                emit_tp(*pend_tp.pop(0))
        emit_o(H - 2)
        emit_tp(*pend_tp.pop(0))
        emit_o(H - 1)
        while pend_tp:
            emit_tp(*pend_tp.pop(0))

        # ---- O-proj + residual, interleaved with LN2 stats so the PE moves
        # straight from wo matmuls into stat matmuls while DVE finalizes ----

        def wo_pass(t0, t1):
            n = t1 - t0
            for fc in range(4):
                if fc < 2:
                    ps = psA.tile([128, 512], F32, tag="mm",
                                  name=f"op{l}_{fc}_{t0}")
                else:
                    ps = psM.tile([128, 512], F32, tag=f"g{fc}",
                                  name=f"op{l}_{fc}_{t0}")
                for c in range(4):
                    nc.tensor.matmul(ps[:, 0:n], wo_t[:, c, 128 * fc:128 * fc + 128],
                                     attnT[:, c, t0:t1], start=(c == 0), stop=(c == 3))
                nc.vector.tensor_add(out=hT[:, fc, t0:t1], in0=hT[:, fc, t0:t1],
                                     in1=ps[:, 0:n])

        TPl = ((NNF, 346), TP[1]) if l == L - 1 else TP
        wo_pass(*TPl[0])
        wo_pass(*TPl[1])
        sm2 = [ln_stats_mm(*TPl[0]), ln_stats_mm(*TPl[1])]
        mT = mpool.tile([128, 4, NTOK], BF16, tag="m", name=f"mT{l}")
        for i, (t0, t1) in enumerate(TPl):
            mu_b, rs_b = ln_stats_act(t0, t1, *sm2[i], f"g{2 * i}", f"g{2 * i + 1}")
            ln_fin(mT, t0, t1, mu_b, rs_b)
        # preload the gelu table while LN2 finalize / MLP1 matmuls run
        dmy = sp.tile([1, 346], F32, tag="var0", name=f"dmy{l}")
        nc.scalar.activation(dmy[0:1, 0:1], mT[0:1, 0, 0:1], AF.Gelu)
        if l < L - 1:
            aT_next = apool.tile([128, 4, NTOK], BF16, tag="a", name=f"aT{l + 1}")
        sm1 = [None, None]
        MP = (((NNF, 436), (436, NTOK)) if l == L - 1
              else ((0, 256), (256, 512), (512, NTOK)))
        for pi, (t0, t1) in enumerate(MP):
            n = t1 - t0
            # two down-proj accumulators, 2 output chunks each (g0: fc0/1,
            # g1: fc2/3) -- frees g2/g3 for the ups rotation below
            acc = [psM.tile([128, 512], F32, tag=f"g{i}",
                            name=f"acc{l}_{t0}_{i}") for i in range(4)]
            ug2s = [None] * 8
            for pr in range(9):
                if pr < 8:
                    # ups for uc pair (2*pr, 2*pr+1) share one PSUM tile so
                    # one gelu call covers both
                    ups = psA.tile([128, 2, 256], F32, tag="mm",
                                   name=f"u{l}_{t0}_{pr}")
                    for ui in range(2):
                        uc = 2 * pr + ui
                        for c in range(4):
                            nc.tensor.matmul(ups[:, ui, 0:n],
                                             wfc_t[:, c, 128 * uc:128 * uc + 128],
                                             mT[:, c, t0:t1],
                                             start=(c == 0), stop=(c == 3))
                    ug2 = up.tile([128, 2, 256], FP8, tag="ug",
                                  name=f"ug{l}_{t0}_{pr}")
                    nc.scalar.activation(ug2[:, :, 0:n], ups[:, :, 0:n],
                                         AF.Gelu, scale=1.0 / 16.0)
                    ug2s[pr] = ug2
                if pr >= 1:
                    p = pr - 1
                    for fc in range(4):
                        nc.tensor.matmul(acc[fc][:, 0:n],
                                         wpr_t[:, 2 * p:2 * p + 2,
                                               128 * fc:128 * fc + 128],
                                         ug2s[p][:, :, 0:n],
                                         start=(p == 0), stop=(p == 7),
                                         perf_mode=mybir.MatmulPerfMode.DoubleRow)
            for fc in range(4):
                nc.vector.tensor_add(out=hT[:, fc, t0:t1], in0=hT[:, fc, t0:t1],
                                     in1=acc[fc][:, 0:n])
            if l < L - 1 and pi >= 1:
                # next layer's LN1 stat matmuls ride the tail of this pass --
                # TP[0] (tokens 0:346) is final once MP pass 1 (through 512)
                # has landed, TP[1] after the last pass. (No ACT here, so the
                # gelu table set is undisturbed.)
                sm1[pi - 1] = ln_stats_mm(*TP[pi - 1])
            if l == L - 1:
                # final-layer logits ride each pass (tokens are final here)
                emit_logits((0, 1) if pi == 0 else (2, 3))
        if l < L - 1:
            # Ln/Exp both live on the exp table set: this also switches the
            # table back for the next layer's attention exponentials
            for pi, (t0, t1) in enumerate(TP):
                mu_b, rs_b = ln_stats_act(t0, t1, *sm1[pi],
                                          f"g{2 * pi}", f"g{2 * pi + 1}")
                ln_fin(aT_next, t0, t1, mu_b, rs_b)
            aT = aT_next

    ctx.close()


# (waitsplit embedded so kernel.py is self-contained)
import types as _types
waitsplit_embedded = _types.ModuleType("waitsplit_embedded")


def _split_excess_waits(nc):
    n_split = 0
    for fn in nc.m.functions:
        for bb in fn.blocks:
            insts = list(bb.instructions)
            new_list = []
            changed = False
            for inst in insts:
                si = getattr(inst, "sync_info", None)
                waits = list(si.on_wait) if si is not None and si.on_wait else []
                cap = 2 if isinstance(inst, mybir.InstEventSemaphore) else 1
                if len(waits) > cap:
                    changed = True
                    keep = waits[-cap:]
                    for w in waits[:-cap]:
                        n_split += 1
                        nop = mybir.InstNoOp(
                            name=f"WSPLIT-{n_split}-{inst.name}",
                            engine=inst.engine,
                            ins=[], outs=[],
                            sync_info=mybir.SyncInfo(on_wait=[w], on_update=[]),
                        )
                        try:
                            nop.bass_nofuse = True
                        except Exception:
                            pass
                        new_list.append(nop)
                    inst.sync_info = mybir.SyncInfo(on_wait=keep,
                                                    on_update=list(si.on_update))
                new_list.append(inst)
            if changed:
                try:
                    bb.instructions = new_list
                except Exception:
                    bb.instructions.clear()
                    bb.instructions.extend(new_list)
    return n_split


waitsplit_embedded.split_excess_waits = _split_excess_waits
sys.modules["waitsplit_embedded"] = waitsplit_embedded


# ---------------- host side ----------------

def _sinusoidal_pos_emb(n_pos, d, n=10000.0):
    pos = np.arange(n_pos, dtype=np.float32)[:, None]
    den = np.power(n, 2.0 * np.arange(d // 2, dtype=np.float32) / d).astype(np.float32)
    emb = np.zeros((n_pos, d), dtype=np.float32)
    emb[:, 0::2] = np.sin(pos / den)
    emb[:, 1::2] = np.cos(pos / den)
    return emb


_PROG = None


def kernel(**inputs):
    global _PROG
    import ml_dtypes
    bf16 = ml_dtypes.bfloat16
    x = np.ascontiguousarray(np.asarray(inputs["x"], dtype=np.float32))
    f = np.ascontiguousarray(np.asarray(inputs["f"], dtype=np.float32))
    delim = np.asarray(inputs["frame_delim"], dtype=np.float32)
    wqkv = np.asarray(inputs["wqkv"], dtype=np.float32)
    wo = np.asarray(inputs["wo"], dtype=np.float32)
    wfc = np.asarray(inputs["wfc"], dtype=np.float32)
    wproj = np.asarray(inputs["wproj"], dtype=np.float32)
    pred_w = np.asarray(inputs["pred_w"], dtype=np.float32)

    # this kernel folds away the (identity) LN affine and (zero) biases;
    # verify that assumption against the actual inputs
    assert np.all(np.asarray(inputs["ln1_g"]) == 1), "nonconst ln1_g"
    assert np.all(np.asarray(inputs["ln2_g"]) == 1), "nonconst ln2_g"
    assert np.all(np.asarray(inputs["ln1_b"]) == 0), "nonzero ln1_b"
    assert np.all(np.asarray(inputs["ln2_b"]) == 0), "nonzero ln2_b"
    for bname in ("bqkv", "bo", "bfc", "bproj"):
        assert np.all(np.asarray(inputs[bname]) == 0), f"nonzero {bname}"

    d2 = np.broadcast_to(delim, (B, N, 1, W))
    fx = np.concatenate([x, d2, f, d2], axis=-2).reshape(B, S, W)
    fx = fx + _sinusoidal_pos_emb(S, W)[None]

    nf_idx = (np.arange(N)[:, None] * BLK + (F + np.arange(T + 2))[None, :]).reshape(-1)
    jj = np.arange(NNF) // (T + 2)
    rr = np.arange(NNF) % (T + 2)
    mask = np.full((NNF, N), NEG, np.float32)
    for i in range(N):
        allowed = ((rr <= T) & (jj <= i)) | ((rr == T + 1) & (jj == i - 1))
        mask[allowed, i] = 0.0

    fp8 = ml_dtypes.float8_e4m3
    # residual stream is carried as 16*h on-chip: h0 and the two residual-
    # producing projections (wo, wproj) are scaled by 16, and the logit
    # weights by 1/16. LN is scale-invariant, so everything else is unchanged.
    wqkvT = np.ascontiguousarray(wqkv.transpose(0, 2, 1).astype(bf16))
    woT = np.ascontiguousarray((wo * 16.0).transpose(0, 2, 1).astype(bf16))
    # wfc additionally x16 purely for fp8 range (folded back in gelu's scale)
    wfcT = np.ascontiguousarray((wfc * 16.0).transpose(0, 2, 1).astype(bf16))
    wprojT = np.ascontiguousarray((wproj * 16.0).transpose(0, 2, 1).astype(fp8))
    predT = np.ascontiguousarray(pred_w.T / 16.0)

    cvec = np.zeros((128, 12), np.float32)
    cvec[:, 0] = 1.0 / W
    cvec[:, 1] = 1.0
    cvec[:, 2] = EPS
    cvec[:, 4:12] = 1.0
    crow = np.ones((1, 128), np.float32)

    if _PROG is None:
        _PROG = build_program()
    nc = _PROG

    in_maps = []
    for c in range(8):
        b, slot = c // 4, c % 4
        cf = CORE_FRAMES[slot]
        fr_idx = np.concatenate([np.arange(i * BLK, i * BLK + F) for i in cf])
        tok = np.concatenate([nf_idx, fr_idx])
        h0T = np.ascontiguousarray(fx[b, tok, :].T * 16.0)
        in_maps.append({
            "h0": h0T,
            "wqkvT": wqkvT, "woT": woT, "wfcT": wfcT, "wprojT": wprojT,
            "predT": predT,
            "nfmask": np.ascontiguousarray(np.repeat(mask[:, cf], F, axis=1)),
            "cvec": cvec, "crow": crow,
        })

    res = run_bass_kernel_spmd(nc, in_maps, list(range(8)))

    out = np.zeros((B, N, F, 1024), np.float32)
    for c in range(8):
        b, slot = c // 4, c % 4
        lo = res.results[c]["logits"].reshape(4, F, 1024)
        for si, i in enumerate(CORE_FRAMES[slot]):
            if slot == 3 and si == 0:
                continue
            out[b, i] = lo[si]
    return out.reshape(B, N * F, 1024)
